# revision 6
# baseline (speedup 1.0000x reference)
"""GIN message-passing via TensorE matmul segment-sum on 8 trn2 cores.

out[f, dst] = sum_slots msg[slot, f] * sel[slot, dst]

 - Slots = edges + self-loops, dst-rank-major, padded per 128-dst tile to a
   uniform G_t slots per node (degree-sorted tiles keep padding ~2%).
 - 128-slot blocks: lhsT (stationary) = msg block [128 slots, 128 feat] bf16;
   rhs (moving) = a tiny static 0/1 selection pattern [128 slots, N_b dst]
   keyed by (G_t, phase) and resident in SBUF; out accumulates into a PSUM
   window [128 feat, 512 dst] (4 tiles), zero-initialized by a K=1 matmul.
 - ACT evacuates each window PSUM f32 -> SBUF bf16; DMA writes [f, dst] out;
   host transposes/unpermutes and upcasts to f32.
"""

import sys

if "/opt/trn_rl_repo" not in sys.path:
    sys.path.insert(0, "/opt/trn_rl_repo")

import numpy as np

N_NODES = 100000
N_EDGES = 1600000
D = 128
N_CORES = 8
SHARD = N_NODES // N_CORES          # 12500
P = 128
NT = (SHARD + P - 1) // P           # 98 tiles per core
PAD = NT * P                        # 12544
WTILES = 4                          # tiles per psum window (512 dst)
NWIN = (NT + WTILES - 1) // WTILES  # 25

_nc_cache = {}


def _pattern_layout(G):
    """Distinct (G_t, phase) selection patterns and their column offsets."""
    pats = {}
    cols = 0
    for t in range(NT):
        g = int(G[t])
        for j in range(g):
            phi = (128 * j) % g
            if (g, phi) not in pats:
                n = (phi + 127) // g + 1
                pats[(g, phi)] = (cols, n)
                cols += n
    return pats, cols


def _build(G, repeat=1):
    import concourse.bacc as bacc
    import concourse.tile as tile
    from concourse import mybir

    nc = bacc.Bacc("TRN2", target_bir_lowering=False, debug=False,
                   num_devices=N_CORES)
    W = int(sum(G))
    woff = np.concatenate([[0], np.cumsum(G)]).astype(np.int64)
    pats, patcols = _pattern_layout(G)

    msgs_d = nc.dram_tensor("msgs", [P, W * D], mybir.dt.bfloat16,
                            kind="ExternalInput").ap()
    pat_d = nc.dram_tensor("pat", [P, patcols], mybir.dt.bfloat16,
                           kind="ExternalInput").ap()
    out_d = nc.dram_tensor("out", [P, PAD], mybir.dt.bfloat16,
                           kind="ExternalOutput").ap()

    with tile.TileContext(nc) as tc:
        with tc.tile_pool(name="const", bufs=1) as constp, \
             tc.tile_pool(name="msgs", bufs=3) as msgsp, \
             tc.tile_pool(name="outs", bufs=2) as outsp, \
             tc.psum_pool(name="acc", bufs=2) as accp:
            pat_t = constp.tile([P, patcols], mybir.dt.bfloat16)
            nc.sync.dma_start(pat_t[:], pat_d[:])

            for _rep in range(repeat):
                for w in range(NWIN):
                    t0, t1 = w * WTILES, min((w + 1) * WTILES, NT)
                    ndst = (t1 - t0) * P
                    cw = int(woff[t1] - woff[t0])       # slot cols this window
                    m = msgsp.tile([P, cw * D], mybir.dt.bfloat16, tag="m")
                    nc.sync.dma_start(m[:], msgs_d[:, woff[t0] * D:woff[t1] * D])

                    pw = accp.tile([P, 512], mybir.dt.float32, tag="pw")
                    # zero-fill on the otherwise-idle DVE (frees PE cycles)
                    nc.vector.memset(pw[:, :ndst], 0.0)

                    nblk = 0
                    for t in range(t0, t1):
                        g = int(G[t])
                        tcol = (t - t0) * P             # psum col base of tile
                        for j in range(g):
                            phi = (128 * j) % g
                            r0 = (128 * j) // g
                            poff, nb = pats[(g, phi)]
                            last = (t == t1 - 1) and (j == g - 1)
                            nc.tensor.matmul(
                                out=pw[:, tcol + r0:tcol + r0 + nb],
                                lhsT=m[:, nblk * D:(nblk + 1) * D],
                                rhs=pat_t[:, poff:poff + nb],
                                start=False, stop=last,
                                skip_group_check=True)
                            nblk += 1

                    o = outsp.tile([P, ndst], mybir.dt.bfloat16, tag="o")
                    nc.scalar.copy(out=o[:], in_=pw[:, :ndst])
                    nc.scalar.dma_start(out_d[:, t0 * P:t0 * P + ndst], o[:])
    nc.compile()
    return nc


def _host_prep(feat, src, dst):
    """Shard + degree-sort + materialize per-core bf16 slot-block streams."""
    from concourse import mybir
    bf16 = mybir.dt.np(mybir.dt.bfloat16)

    deg = np.bincount(dst, minlength=N_NODES)

    order = np.argsort(dst, kind="stable")
    dst_s = dst[order]
    src_s = src[order]
    starts = np.searchsorted(dst_s, np.arange(N_NODES))
    slot = np.arange(N_EDGES, dtype=np.int64) - starts[dst_s]

    perms = []
    Gcs = []
    for c in range(N_CORES):
        degp = deg[c * SHARD:(c + 1) * SHARD] + 1          # +1 self-loop
        perm = np.argsort(-degp, kind="stable")
        perms.append(perm)
        sd = np.concatenate([degp[perm], np.zeros(PAD - SHARD, np.int64)])
        Gcs.append(sd[::P])
    G = np.maximum(np.max(np.stack(Gcs), axis=0), 1)       # [NT]
    woff = np.concatenate([[0], np.cumsum(G)]).astype(np.int64)
    W = int(G.sum())

    slot_src = np.full((N_CORES, P, W), N_NODES, np.int32)
    for c in range(N_CORES):
        base = c * SHARD
        rank = np.empty(SHARD, np.int64)
        rank[perms[c]] = np.arange(SHARD)
        a = np.searchsorted(dst_s, base)
        b = np.searchsorted(dst_s, base + SHARD)
        r = rank[dst_s[a:b] - base]
        slot_src[c, r & (P - 1), woff[r >> 7] + slot[a:b]] = src_s[a:b]
        rs = rank
        slot_src[c, rs & (P - 1), woff[rs >> 7] + deg[base:base + SHARD]] = (
            base + np.arange(SHARD))

    feat_bf = np.zeros((N_NODES + 1, D), bf16)
    feat_bf[:N_NODES] = feat.astype(bf16)

    # flat slot order: tile-major, then rank-in-tile, then slot g
    strms = []
    for c in range(N_CORES):
        flat = np.concatenate(
            [slot_src[c][:, woff[t]:woff[t + 1]].reshape(-1)
             for t in range(NT)])
        rows = flat.reshape(W, P)                          # [block, k]
        blk = feat_bf[rows]                                # [block, k, f]
        strms.append(np.ascontiguousarray(
            blk.transpose(1, 0, 2).reshape(P, W * D)))

    # pattern table
    pats, patcols = _pattern_layout(G)
    pat = np.zeros((P, patcols), bf16)
    for (g, phi), (off, nb) in pats.items():
        k = np.arange(P)
        pat[k, off + (phi + k) // g] = 1.0

    return strms, pat, perms, tuple(int(g) for g in G)


LAST_RUN = None


def kernel(feat, src, dst):
    global LAST_RUN
    feat = np.ascontiguousarray(np.asarray(feat), dtype=np.float32)
    src = np.asarray(src).astype(np.int64)
    dst = np.asarray(dst).astype(np.int64)
    assert feat.shape == (N_NODES, D) and src.shape == (N_EDGES,)

    strms, pat, perms, G = _host_prep(feat, src, dst)

    if G not in _nc_cache:
        _nc_cache[G] = _build(G)
    nc = _nc_cache[G]

    from concourse.bass_utils import run_bass_kernel_spmd

    in_maps = [{"msgs": strms[c], "pat": pat} for c in range(N_CORES)]
    res = run_bass_kernel_spmd(nc, in_maps, core_ids=list(range(N_CORES)))
    LAST_RUN = res

    out = np.empty((N_NODES, D), np.float32)
    for c in range(N_CORES):
        oc = np.asarray(res.results[c]["out"])             # [f, PAD] bf16
        ocr = oc.T.astype(np.float32)                      # [PAD, f]
        out[c * SHARD:(c + 1) * SHARD][perms[c]] = ocr[:SHARD]
    return out
